# revision 4
# baseline (speedup 1.0000x reference)
"""Trainium2 Bass kernel for causal attention + proj + causal_features.

Problem shapes: x [2, 2048, 1024], H=16 heads, HD=64.
Strategy (8 NeuronCores):
  - Head-parallel attention: core i computes QKV + attention for heads {2i, 2i+1}
    over both batches, everything in transposed [channel, token] layout so the
    contraction dim sits on SBUF partitions.
  - Scores computed as S^T[k, q] = K @ Q^T (contraction d=64); exp on ScalarE
    (scale=1/8 folded in); causal mask applied only on diagonal-band tiles via
    a precomputed 0/1 multiplier; upper-triangle tiles skipped entirely.
  - attn @ V via lhsT = [V | 1] (ones column) so the softmax denominators fall
    out of the same matmul as row 64 of the PSUM accumulator.
  - AllToAll reshards attention output from head-parallel to token-parallel;
    each core then computes proj + bias and causal_features for its 512-token
    slice. Outputs returned transposed [1024, 512] and re-assembled on host.
  - All matmuls run in float32r (full PE rate at free dim >= 256, ~1e-4 rel err).
"""

import numpy as np
import concourse.bass as bass
import concourse.mybir as mybir
import concourse.tile as tile
from concourse import bacc
from concourse.bass_utils import run_bass_kernel_spmd

B, N, C, H, HD = 2, 2048, 1024, 16, 64
NCORES = 8
TOK = B * N            # 4096 global tokens
TPC = TOK // NCORES    # 512 tokens per core (output slice)
QC = 512               # q chunk width
KTILE = 128            # k tile height
NKT = N // KTILE       # 16 k tiles per batch
NQC = N // QC          # 4 q chunks per batch
f32 = mybir.dt.float32
f32r = mybir.dt.float32r
AF = mybir.ActivationFunctionType
ALU = mybir.AluOpType

_CACHE = {}


def _build_program():
    nc = bacc.Bacc("TRN2", target_bir_lowering=False, debug=False, num_devices=NCORES)

    xT_d = nc.dram_tensor("xT", [C, TOK], f32r, kind="ExternalInput")
    wq_d = nc.dram_tensor("wq", [C, 128], f32r, kind="ExternalInput")
    wk_d = nc.dram_tensor("wk", [C, 128], f32r, kind="ExternalInput")
    wv_d = nc.dram_tensor("wv", [C, 128], f32r, kind="ExternalInput")
    wp_d = nc.dram_tensor("wp", [C, C], f32r, kind="ExternalInput")
    wc_d = nc.dram_tensor("wc", [C, C], f32r, kind="ExternalInput")
    bp_d = nc.dram_tensor("bp", [C], f32, kind="ExternalInput")
    bc_d = nc.dram_tensor("bc", [C], f32, kind="ExternalInput")
    maskt_d = nc.dram_tensor("maskt", [128, 4 * QC], f32, kind="ExternalInput")
    ident_d = nc.dram_tensor("ident", [128, 64], f32r, kind="ExternalInput")
    outT_d = nc.dram_tensor("outT", [C, TPC], f32r, kind="ExternalOutput")
    czT_d = nc.dram_tensor("czT", [C, TPC], f32r, kind="ExternalOutput")

    with tile.TileContext(nc) as tc:
        with tc.tile_pool(name="sb", bufs=1) as sb, \
             tc.tile_pool(name="ps", bufs=1, space="PSUM") as ps, \
             tc.tile_pool(name="dr", bufs=1, space="DRAM") as dr:

            # ---- constants ----
            wqkv_sb = sb.tile([128, 3 * 8 * 128], f32r)     # [c_in 128][m(q/k/v)*8 + kt]*128
            for m, wd in enumerate((wq_d, wk_d, wv_d)):
                for kt in range(8):
                    nc.sync.dma_start(
                        wqkv_sb[:, (m * 8 + kt) * 128:(m * 8 + kt + 1) * 128],
                        wd[kt * 128:(kt + 1) * 128, :])
            maskt_sb = sb.tile([128, 4 * QC], f32)
            nc.sync.dma_start(maskt_sb[:], maskt_d[:])
            ident_sb = sb.tile([128, 64], f32r)
            nc.sync.dma_start(ident_sb[:], ident_d[:])
            bias_sb = sb.tile([128, 16], f32)               # cols 0:8 = bp tiles, 8:16 = bc
            for m in range(8):
                nc.sync.dma_start(bias_sb[:, m:m + 1],
                                  bp_d[m * 128:(m + 1) * 128].rearrange("(p o) -> p o", o=1))
                nc.sync.dma_start(bias_sb[:, 8 + m:9 + m],
                                  bc_d[m * 128:(m + 1) * 128].rearrange("(p o) -> p o", o=1))

            qT_sb = sb.tile([128, TOK], f32r)   # rows: head_local*64 + d
            kT_sb = sb.tile([128, TOK], f32r)
            # rotating 16KB scratch: vT -> outT -> pj -> cz
            vT_sb = sb.tile([128, TOK], f32r, tag="scratch16", bufs=2)

            # ---- QKV (transposed): [q/k/v]T[c_out 128, tok] = W.T @ x.T ----
            for n in range(TOK // QC):
                xts = []
                for kt in range(8):
                    xt = sb.tile([128, QC], f32r, tag="xt", bufs=10, name=f"xt_{n}_{kt}")
                    nc.sync.dma_start(xt[:], xT_d[kt * 128:(kt + 1) * 128, n * QC:(n + 1) * QC])
                    xts.append(xt)
                for m, dst in enumerate((qT_sb, kT_sb, vT_sb)):
                    acc = ps.tile([128, QC], f32, tag="mm_ps", bufs=2, name=f"qkv_ps_{n}_{m}")
                    for kt in range(8):
                        nc.tensor.matmul(acc[:], wqkv_sb[:, (m * 8 + kt) * 128:(m * 8 + kt + 1) * 128],
                                         xts[kt][:], start=(kt == 0), stop=(kt == 7))
                    if m == 0:
                        nc.scalar.activation(dst[:, n * QC:(n + 1) * QC], acc[:], AF.Copy)
                    else:
                        nc.vector.tensor_copy(dst[:, n * QC:(n + 1) * QC], acc[:])

            # ---- V transpose: vT [2h x 64, tok] -> v_aug [tok-tile 128, 65] per (b,h,kt) ----
            v_aug = sb.tile([128, B * 2 * NKT * 65], f32r)
            for b in range(B):
                for h in range(2):
                    for kt in range(NKT):
                        tp = ps.tile([128, 64], f32r, tag="mm_ps", bufs=2, name=f"vt_{b}_{h}_{kt}")
                        nc.tensor.transpose(
                            tp[:], vT_sb[h * 64:(h + 1) * 64,
                                         b * N + kt * 128:b * N + (kt + 1) * 128],
                            ident_sb[h * 64:(h + 1) * 64, :])
                        slot = ((b * 2 + h) * NKT + kt) * 65
                        nc.vector.tensor_copy(v_aug[:, slot:slot + 64], tp[:])
            # ones column for every (b,h,kt) slot in one strided write
            nc.vector.tensor_scalar(v_aug[:, 64::65], maskt_sb[:, 0:B * 2 * NKT],
                                    0.0, 1.0, ALU.mult, ALU.add)

            # ---- attention (causal), heads A/B on partition halves ----
            outT_sb = sb.tile([128, TOK], f32r, tag="scratch16", bufs=2)
            for b in range(B):
                for j in range(NQC):
                    qc0 = b * N + j * QC
                    avs = []
                    for h in range(2):
                        av = ps.tile([65, QC], f32, tag=f"av{h}", bufs=1, name=f"av_{b}_{j}_{h}")
                        avs.append(av)
                    last_kt = 4 * j + 3
                    for g in range(2 * j + 2):
                        kt0 = 2 * g
                        for h in range(2):
                            sc2 = ps.tile([128, 2 * QC], f32, tag="sc", bufs=2,
                                          name=f"sc_{b}_{j}_{g}_{h}")
                            for t in range(2):
                                kk = b * N + (kt0 + t) * 128
                                nc.tensor.matmul(sc2[:, t * QC:(t + 1) * QC],
                                                 kT_sb[h * 64:(h + 1) * 64, kk:kk + 128],
                                                 qT_sb[h * 64:(h + 1) * 64, qc0:qc0 + QC],
                                                 start=True, stop=True)
                            ex2 = sb.tile([128, 2 * QC], f32r, tag="ex", bufs=3,
                                          name=f"ex_{b}_{j}_{g}_{h}")
                            nc.scalar.activation(ex2[:], sc2[:], AF.Exp, scale=HD ** -0.5)
                            if kt0 >= 4 * j:
                                off = (kt0 - 4 * j) * QC
                                nc.vector.tensor_tensor(ex2[:], ex2[:],
                                                        maskt_sb[:, off:off + 2 * QC], ALU.mult)
                            for t in range(2):
                                kt = kt0 + t
                                slot = ((b * 2 + h) * NKT + kt) * 65
                                nc.tensor.matmul(avs[h][:], v_aug[:, slot:slot + 65],
                                                 ex2[:, t * QC:(t + 1) * QC],
                                                 start=(kt == 0), stop=(kt == last_kt))
                    for h in range(2):
                        rc = sb.tile([1, QC], f32, tag="rc", bufs=2, name=f"rc_{b}_{j}_{h}")
                        nc.vector.reciprocal(rc[:], avs[h][64:65, :])
                        rb = sb.tile([64, QC], f32, tag="rb", bufs=2, name=f"rb_{b}_{j}_{h}")
                        nc.gpsimd.partition_broadcast(rb[:], rc[:])
                        nc.vector.tensor_tensor(outT_sb[h * 64:(h + 1) * 64, qc0:qc0 + QC],
                                                avs[h][0:64, :], rb[:], ALU.mult)

            # ---- AllToAll: head-parallel [128 c, 4096 tok] -> token-parallel [1024 c, 512 tok] ----
            a2a_in = dr.tile([NCORES, 128, TPC], f32r)
            a2a_out = dr.tile([NCORES, 128, TPC], f32r)
            for d in range(NCORES):
                nc.sync.dma_start(a2a_in[d], outT_sb[:, d * TPC:(d + 1) * TPC])
            nc.gpsimd.collective_compute("AllToAll", ALU.bypass,
                                         replica_groups=[list(range(NCORES))],
                                         ins=[a2a_in.opt()], outs=[a2a_out.opt()])
            otf = sb.tile([128, NCORES * TPC], f32r, tag="scratch16", bufs=2)   # [c_in 128][src core] = full attn out, T
            for d in range(NCORES):
                nc.sync.dma_start(otf[:, d * TPC:(d + 1) * TPC], a2a_out[d])

            # ---- proj + bias (projT [c_out, tok]) ----
            pj_sb = sb.tile([128, C // 128 * TPC], f32r, tag="scratch16", bufs=2)
            for m in range(8):
                wpm = sb.tile([128, 1024], f32r, tag="wp", bufs=3, name=f"wpm_{m}")
                for kt in range(8):
                    nc.sync.dma_start(wpm[:, kt * 128:(kt + 1) * 128],
                                      wp_d[kt * 128:(kt + 1) * 128, m * 128:(m + 1) * 128])
                acc = ps.tile([128, TPC], f32, tag=f"av{m % 2}", bufs=1, name=f"pj_ps_{m}")
                for kt in range(8):
                    nc.tensor.matmul(acc[:], wpm[:, kt * 128:(kt + 1) * 128],
                                     otf[:, kt * TPC:(kt + 1) * TPC],
                                     start=(kt == 0), stop=(kt == 7))
                nc.vector.tensor_scalar_add(pj_sb[:, m * TPC:(m + 1) * TPC], acc[:],
                                            bias_sb[:, m:m + 1])
                nc.sync.dma_start(outT_d[m * 128:(m + 1) * 128, :], pj_sb[:, m * TPC:(m + 1) * TPC])

            # ---- causal_features + bias ----
            cz_sb = sb.tile([128, C // 128 * TPC], f32r, tag="scratch16", bufs=2)
            for m in range(8):
                wcm = sb.tile([128, 1024], f32r, tag="wp", bufs=3, name=f"wcm_{m}")
                for kt in range(8):
                    nc.sync.dma_start(wcm[:, kt * 128:(kt + 1) * 128],
                                      wc_d[kt * 128:(kt + 1) * 128, m * 128:(m + 1) * 128])
                acc = ps.tile([128, TPC], f32, tag=f"av{m % 2}", bufs=1, name=f"cz_ps_{m}")
                for kt in range(8):
                    nc.tensor.matmul(acc[:], wcm[:, kt * 128:(kt + 1) * 128],
                                     pj_sb[:, kt * TPC:(kt + 1) * TPC],
                                     start=(kt == 0), stop=(kt == 7))
                nc.vector.tensor_scalar_add(cz_sb[:, m * TPC:(m + 1) * TPC], acc[:],
                                            bias_sb[:, 8 + m:9 + m])
                nc.sync.dma_start(czT_d[m * 128:(m + 1) * 128, :], cz_sb[:, m * TPC:(m + 1) * TPC])

    nc.finalize()
    return nc


def _host_inputs(x, mask, W_qkv, W_proj, b_proj, W_causal, b_causal):
    x = np.asarray(x, dtype=np.float32)
    xT = np.ascontiguousarray(x.reshape(TOK, C).T)
    m2 = np.asarray(mask).reshape(N, N)
    # diagonal-band mask multiplier tiles in S^T [k, q] layout, offsets d0 = k0-q0
    q0 = N - QC
    tiles = []
    for d0 in (0, 128, 256, 384):
        k0 = q0 + d0
        tiles.append(np.ascontiguousarray(m2[q0:q0 + QC, k0:k0 + 128].T.astype(np.float32)))
    maskt = np.concatenate(tiles, axis=1)
    ident = np.ascontiguousarray(np.concatenate([np.eye(64, dtype=np.float32)] * 2, axis=0))
    W_qkv = np.asarray(W_qkv, dtype=np.float32)
    shared = {
        "xT": xT,
        "wp": np.asarray(W_proj, dtype=np.float32),
        "wc": np.asarray(W_causal, dtype=np.float32),
        "bp": np.asarray(b_proj, dtype=np.float32),
        "bc": np.asarray(b_causal, dtype=np.float32),
        "maskt": maskt,
        "ident": ident,
    }
    in_maps = []
    for i in range(NCORES):
        im = dict(shared)
        im["wq"] = np.ascontiguousarray(W_qkv[:, i * 128:(i + 1) * 128])
        im["wk"] = np.ascontiguousarray(W_qkv[:, C + i * 128:C + (i + 1) * 128])
        im["wv"] = np.ascontiguousarray(W_qkv[:, 2 * C + i * 128:2 * C + (i + 1) * 128])
        in_maps.append(im)
    return in_maps


def kernel(x, mask, W_qkv, W_proj, b_proj, W_causal, b_causal):
    if "nc" not in _CACHE:
        _CACHE["nc"] = _build_program()
    nc = _CACHE["nc"]
    in_maps = _host_inputs(x, mask, W_qkv, W_proj, b_proj, W_causal, b_causal)
    res = run_bass_kernel_spmd(nc, in_maps, list(range(NCORES)))
    out = np.empty((TOK, C), dtype=np.float32)
    cz = np.empty((TOK, C), dtype=np.float32)
    for i in range(NCORES):
        out[i * TPC:(i + 1) * TPC, :] = res.results[i]["outT"].T
        cz[i * TPC:(i + 1) * TPC, :] = res.results[i]["czT"].T
    return (out.reshape(B, N, C), cz.reshape(B, N, C))
